# revision 3
# baseline (speedup 1.0000x reference)
"""Tropical max-plus 2D conv (BroadcastConv tropical_max) on 8 Trainium2 cores.

out[b,o,y,x] = max_{c,i,j} img_pad[b,c,y+i,x+j] + kflip[o,c,i,j]
  imgs [4,32,128,128] f32, kernel [32,32,5,5] f32, stride=1, pad=2, dil=1.

Sharding: output channels O=32 split across 8 cores (4 per core); every core
keeps the full batch so the DVE instruction free-dim is long (2048 elems).

Per-core layout:
  partitions p = o_local*32 + ys   (o_local in [0,4), ys = y % 32)
  free       = (b:4, yb:4, x)      (yb = y // 32, so y = yb*32 + ys)
The 5 vertical kernel taps are handled by 5 pre-shifted SBUF copies of each
input channel (DMA'd with a row offset into a persistently -inf-padded tile);
the 5 horizontal taps are free-dim column offsets into x-padding. Each
(i,c,j) tap is then one fused DVE scalar_tensor_tensor:
  acc = max(shifted_img + k[o,c,i,j], acc)
with the k value as a per-partition [128,1] scalar operand (k varies over the
o_local partition groups). 800 such instructions per core, FD=2048.
"""

import numpy as np

NCORES = 8
B, C, H, W = 4, 32, 128, 128
O, KH, KW = 32, 5, 5
OL = O // NCORES  # 4 output channels per core
PAD = 2
YS, YB = 32, 4  # y = yb*YS + ys
XX = W + 2 * PAD  # 132 (x-padded row)
NK = KH * C * KW  # 800 scalar-table entries per o_local
NEG = float("-inf")

_CACHE = {}


def _build_program():
    import concourse.mybir as mybir
    from concourse import bacc
    from concourse.tile import TileContext

    f32 = mybir.dt.float32
    nc = bacc.Bacc("TRN2", target_bir_lowering=False)
    # imgsr: host-prepped [c, ys, b, yb, xx] with x-padding (-inf) baked in,
    # so every tile-load DMA merges to <=3 dims.
    imgs_d = nc.declare_dram_parameter("imgsr", [C, YS, B, YB, XX], f32, isOutput=False)
    kprep_d = nc.declare_dram_parameter("kprep", [128, NK], f32, isOutput=False)
    out_d = nc.declare_dram_parameter("out", [OL, YS, B, YB, W], f32, isOutput=True)

    NBUF = 3  # manual multi-buffering depth per shift-pool

    with TileContext(nc) as tc:
        with tc.tile_pool(name="sbuf", bufs=1) as pool:
            k_sb = pool.tile([128, NK], f32, tag="ksb", name="ksb")
            acc = pool.tile([128, B, YB, W], f32, tag="acc", name="acc")
            tiles = [
                [
                    pool.tile([128, B, YB, XX], f32, tag=f"T{i}_{bi}", name=f"T{i}_{bi}")
                    for bi in range(NBUF)
                ]
                for i in range(KH)
            ]

            nc.sync.dma_start(out=k_sb[:], in_=kprep_d[:])
            # One-time -inf fill: pad rows/cols stay -inf forever because the
            # per-(c,i) interior DMAs below always write the same region.
            for i in range(KH):
                for bi in range(NBUF):
                    nc.gpsimd.memset(tiles[i][bi][:], NEG)
            nc.vector.memset(acc[:], NEG)

            rv = imgs_d  # [c, ys, b, yb, xx]

            for ci in range(C):
                for i in range(KH):
                    t = tiles[i][ci % NBUF]
                    d = i - PAD
                    # Load rows r = 32*yb + ys + d of channel ci into
                    # o_local group 0 (partitions 0..32), interior cols.
                    if d == 0:
                        nc.sync.dma_start(out=t[0:YS], in_=rv[ci])
                    elif d < 0:
                        nc.sync.dma_start(out=t[-d:YS], in_=rv[ci, 0 : YS + d])
                        nc.sync.dma_start(
                            out=t[0:-d, :, 1:YB, :],
                            in_=rv[ci, YS + d : YS, :, 0 : YB - 1, :],
                        )
                    else:
                        nc.sync.dma_start(out=t[0 : YS - d], in_=rv[ci, d:YS])
                        nc.sync.dma_start(
                            out=t[YS - d : YS, :, 0 : YB - 1, :],
                            in_=rv[ci, 0:d, :, 1:YB, :],
                        )
                    # Replicate group 0 into the other 3 o_local groups
                    # (SBUF->SBUF DMA; pads are -inf in the source too).
                    for g in range(1, OL):
                        nc.gpsimd.dma_start(
                            out=t[g * YS : (g + 1) * YS], in_=t[0:YS]
                        )
                for i in range(KH):
                    t = tiles[i][ci % NBUF]
                    for j in range(KW):
                        idx = (i * C + ci) * KW + j
                        nc.vector.scalar_tensor_tensor(
                            out=acc[:],
                            in0=t[:, :, :, j : j + W],
                            scalar=k_sb[:, idx : idx + 1],
                            in1=acc[:],
                            op0=mybir.AluOpType.add,
                            op1=mybir.AluOpType.max,
                        )

            for o in range(OL):
                nc.sync.dma_start(
                    out=out_d[o], in_=acc[o * YS : (o + 1) * YS]
                )

    nc.compile()
    return nc


def _get_program():
    if "nc" not in _CACHE:
        _CACHE["nc"] = _build_program()
    return _CACHE["nc"]


def _prep_inputs(imgs, kernel):
    imgs = np.asarray(imgs, dtype=np.float32)
    # [c, ys, b, yb, xx] with -inf x-pad baked in
    imgsr = np.full((C, YS, B, YB, XX), NEG, dtype=np.float32)
    imgsr[:, :, :, :, PAD : PAD + W] = (
        imgs.transpose(1, 2, 0, 3)
        .reshape(C, YB, YS, B, W)
        .transpose(0, 2, 3, 1, 4)
    )
    imgsr = np.ascontiguousarray(imgsr)
    kf = np.asarray(kernel, dtype=np.float32)[:, :, ::-1, ::-1]  # conv flip
    in_maps = []
    for m in range(NCORES):
        sl = kf[OL * m : OL * (m + 1)]  # [OL, C, KH, KW]
        # column index = (i*C + c)*KW + j  ->  order (o, i, c, j)
        tab = np.ascontiguousarray(sl.transpose(0, 2, 1, 3)).reshape(OL, NK)
        kprep = np.repeat(tab, YS, axis=0)  # [128, NK]
        in_maps.append({"imgsr": imgsr, "kprep": np.ascontiguousarray(kprep)})
    return in_maps


def run_spmd(imgs, kernel, trace=False):
    """Run the SPMD program; returns (full_output, BassKernelResults)."""
    from concourse.bass_utils import run_bass_kernel_spmd

    nc = _get_program()
    in_maps = _prep_inputs(imgs, kernel)
    res = run_bass_kernel_spmd(nc, in_maps, list(range(NCORES)), trace=trace)
    full = np.empty((B, O, H, W), dtype=np.float32)
    for m in range(NCORES):
        # per-core out is [OL, YS, B, YB, W] -> [B, OL, YB*YS..., W]
        r = res.results[m]["out"].transpose(2, 0, 3, 1, 4)  # [B, OL, YB, YS, W]
        full[:, OL * m : OL * (m + 1)] = r.reshape(B, OL, H, W)
    return full, res


def kernel(imgs, kernel, stride=1, padding=2, dilation=1, **_ignored):
    assert int(stride) == 1 and int(padding) == 2 and int(dilation) == 1, (
        "kernel compiled for stride=1, padding=2, dilation=1"
    )
    assert tuple(imgs.shape) == (B, C, H, W), imgs.shape
    assert tuple(kernel.shape) == (O, C, KH, KW), kernel.shape
    full, _ = run_spmd(imgs, kernel, trace=False)
    return full


# revision 4
# speedup vs baseline: 1.1299x; 1.1299x over previous
"""Tropical max-plus 2D conv (BroadcastConv tropical_max) on 8 Trainium2 cores.

out[b,o,y,x] = max_{c,i,j} img_pad[b,c,y+i,x+j] + kflip[o,c,i,j]
  imgs [4,32,128,128] f32, kernel [32,32,5,5] f32, stride=1, pad=2, dil=1.

Sharding: output channels O=32 split across 8 cores (4 per core); every core
keeps the full batch so the DVE instruction free-dim is long (2048 elems).

Per-core layout:
  partitions p = o_local*32 + ys   (o_local in [0,4), ys = y % 32)
  free       = (b:4, yb:4, x)      (y = yb*32 + ys)
Host preps imgs into Y2 [c, u:36, b, yb, xx:132] with -inf padding baked into
both the 36 row-slots (u = ys + i covers shifts i in [0,5)) and the x columns,
so each of the 5 vertical kernel taps is ONE rectangular DMA into partition
group 0, replicated to the other 3 o_local groups by two log-step SBUF-to-SBUF
DMAs. The 5 horizontal taps are free-dim column offsets into the x-padding.
Each (i,c,j) tap is then one fused DVE scalar_tensor_tensor instruction:
  acc = max(shifted_img + k[o,c,i,j], acc)
with the k value as a per-partition [128,1] scalar operand (k varies over the
o_local partition groups). 800 such instructions per core, FD=2048; the kernel
is DVE-throughput-bound (fp32 tensor ops are 1 elem/cycle/lane on trn2).
"""

import numpy as np

NCORES = 8
B, C, H, W = 4, 32, 128, 128
O, KH, KW = 32, 5, 5
OL = O // NCORES  # 4 output channels per core
PAD = 2
YS, YB = 32, 4  # y = yb*YS + ys
XX = W + 2 * PAD  # 132 (x-padded row)
YU = YS + 2 * PAD  # 36 padded row-slots (covers ys + shift for all 5 taps)
NK = KH * C * KW  # 800 scalar-table entries per o_local
NEG = float("-inf")

_CACHE = {}


def _build_program():
    import concourse.mybir as mybir
    from concourse import bacc
    from concourse.tile import TileContext

    f32 = mybir.dt.float32
    nc = bacc.Bacc("TRN2", target_bir_lowering=False)
    imgs_d = nc.declare_dram_parameter("imgsr", [C, YU, B, YB, XX], f32, isOutput=False)
    kprep_d = nc.declare_dram_parameter("kprep", [128, NK], f32, isOutput=False)
    out_d = nc.declare_dram_parameter("out", [OL, YS, B, YB, W], f32, isOutput=True)

    NBUF = 4  # multi-buffering depth per shift-pool

    with TileContext(nc) as tc:
        with tc.tile_pool(name="sbuf", bufs=1) as pool:
            k_sb = pool.tile([128, NK], f32, tag="ksb", name="ksb")
            acc = pool.tile([128, B, YB, W], f32, tag="acc", name="acc")
            tiles = [
                [
                    pool.tile([128, B, YB, XX], f32, tag=f"T{i}_{bi}", name=f"T{i}_{bi}")
                    for bi in range(NBUF)
                ]
                for i in range(KH)
            ]

            nc.sync.dma_start(out=k_sb[:], in_=kprep_d[:])
            nc.vector.memset(acc[:], NEG)

            rv = imgs_d  # [c, u, b, yb, xx]

            for ci in range(C):
                for i in range(KH):
                    t = tiles[i][ci % NBUF]
                    # One rectangular load for o_local group 0: partition ys
                    # gets padded row u = ys + i (i.e. image row yb*32+ys+i-2).
                    nc.sync.dma_start(out=t[0:YS], in_=rv[ci, i : i + YS])
                    # Log-step replication into the other 3 o_local groups.
                    nc.gpsimd.dma_start(out=t[YS : 2 * YS], in_=t[0:YS])
                    nc.gpsimd.dma_start(out=t[2 * YS : 4 * YS], in_=t[0 : 2 * YS])
                for i in range(KH):
                    t = tiles[i][ci % NBUF]
                    for j in range(KW):
                        idx = (i * C + ci) * KW + j
                        nc.vector.scalar_tensor_tensor(
                            out=acc[:],
                            in0=t[:, :, :, j : j + W],
                            scalar=k_sb[:, idx : idx + 1],
                            in1=acc[:],
                            op0=mybir.AluOpType.add,
                            op1=mybir.AluOpType.max,
                        )

            for o in range(OL):
                nc.sync.dma_start(out=out_d[o], in_=acc[o * YS : (o + 1) * YS])

    nc.compile()
    return nc


def _get_program():
    if "nc" not in _CACHE:
        _CACHE["nc"] = _build_program()
    return _CACHE["nc"]


def _prep_inputs(imgs, kernel):
    imgs = np.asarray(imgs, dtype=np.float32)
    # fully padded image, -inf ring of width 2
    padded = np.full((B, C, H + 2 * PAD, W + 2 * PAD), NEG, dtype=np.float32)
    padded[:, :, PAD : PAD + H, PAD : PAD + W] = imgs
    # Y2[c, u, b, yb, x] = padded[b, c, 32*yb + u, x]
    rows = 32 * np.arange(YB)[None, :] + np.arange(YU)[:, None]  # [YU, YB]
    y2 = np.ascontiguousarray(padded[:, :, rows, :].transpose(1, 2, 0, 3, 4))
    kf = np.asarray(kernel, dtype=np.float32)[:, :, ::-1, ::-1]  # conv flip
    in_maps = []
    for m in range(NCORES):
        sl = kf[OL * m : OL * (m + 1)]  # [OL, C, KH, KW]
        # column index = (i*C + c)*KW + j  ->  order (o, i, c, j)
        tab = np.ascontiguousarray(sl.transpose(0, 2, 1, 3)).reshape(OL, NK)
        kprep = np.repeat(tab, YS, axis=0)  # [128, NK]
        in_maps.append({"imgsr": y2, "kprep": np.ascontiguousarray(kprep)})
    return in_maps


def run_spmd(imgs, kernel, trace=False):
    """Run the SPMD program; returns (full_output, BassKernelResults)."""
    from concourse.bass_utils import run_bass_kernel_spmd

    nc = _get_program()
    in_maps = _prep_inputs(imgs, kernel)
    res = run_bass_kernel_spmd(nc, in_maps, list(range(NCORES)), trace=trace)
    full = np.empty((B, O, H, W), dtype=np.float32)
    for m in range(NCORES):
        # per-core out is [OL, YS, B, YB, W]
        r = res.results[m]["out"].transpose(2, 0, 3, 1, 4)  # [B, OL, YB, YS, W]
        full[:, OL * m : OL * (m + 1)] = r.reshape(B, OL, H, W)
    return full, res


def kernel(imgs, kernel, stride=1, padding=2, dilation=1, **_ignored):
    assert int(stride) == 1 and int(padding) == 2 and int(dilation) == 1, (
        "kernel compiled for stride=1, padding=2, dilation=1"
    )
    assert tuple(imgs.shape) == (B, C, H, W), imgs.shape
    assert tuple(kernel.shape) == (O, C, KH, KW), kernel.shape
    full, _ = run_spmd(imgs, kernel, trace=False)
    return full


# revision 6
# speedup vs baseline: 774.7185x; 685.6300x over previous
"""Tropical max-plus 2D conv (BroadcastConv tropical_max) on 8 Trainium2 cores.

out[b,o,y,x] = max_{c,i,j} img_pad[b,c,y+i,x+j] + kflip[o,c,i,j]
  imgs [4,32,128,128] f32, kernel [32,32,5,5] f32, stride=1, pad=2, dil=1.

Sharding: output channels O=32 split across 8 cores (4 per core); every core
keeps the full batch so the DVE instruction free-dim is long (2048 elems).

Per-core layout:
  partitions p = o_local*32 + ys   (o_local in [0,4), ys = y % 32)
  free       = (b:4, yb:4, x)      (y = yb*32 + ys)
Host preps imgs into Y2 [c, u:36, b, yb, xx:132] with -inf padding baked into
both the 36 row-slots (u = ys + i covers shifts i in [0,5)) and the x columns,
so each of the 5 vertical kernel taps is ONE rectangular DMA into partition
group 0, replicated to the other 3 o_local groups by parallel SBUF-to-SBUF
DMAs. The 5 horizontal taps are free-dim column offsets into the x-padding.
Each (i,c,j) tap is then one fused DVE scalar_tensor_tensor instruction:
  acc = max(shifted_img + k[o,c,i,j], acc)
with the k value as a per-partition [128,1] scalar operand (k varies over the
o_local partition groups). 800 such instructions per core, FD=2048; the kernel
is DVE-throughput-bound (fp32 tensor ops are 1 elem/cycle/lane on trn2).
"""

import numpy as np

NCORES = 8
B, C, H, W = 4, 32, 128, 128
O, KH, KW = 32, 5, 5
OL = O // NCORES  # 4 output channels per core
PAD = 2
YS, YB = 32, 4  # y = yb*YS + ys
XX = W + 2 * PAD  # 132 (x-padded row)
YU = YS + 2 * PAD  # 36 padded row-slots (covers ys + shift for all 5 taps)
NK = KH * C * KW  # 800 scalar-table entries per o_local
NEG = float("-inf")

_CACHE = {}


def _build_program():
    import concourse.mybir as mybir
    from concourse import bacc
    from concourse.tile import TileContext

    f32 = mybir.dt.float32
    nc = bacc.Bacc("TRN2", target_bir_lowering=False)
    imgs_d = nc.declare_dram_parameter("imgsr", [C, YU, B, YB, XX], f32, isOutput=False)
    kprep_d = nc.declare_dram_parameter("kprep", [128, NK], f32, isOutput=False)
    out_d = nc.declare_dram_parameter("out", [OL, YS, B, YB, W], f32, isOutput=True)

    NBUF = 4  # multi-buffering depth per shift-pool

    with TileContext(nc) as tc:
        with tc.tile_pool(name="sbuf", bufs=1) as pool:
            k_sb = pool.tile([128, NK], f32, tag="ksb", name="ksb")
            acc = pool.tile([128, B, YB, W], f32, tag="acc", name="acc")
            tiles = [
                [
                    pool.tile([128, B, YB, XX], f32, tag=f"T{i}_{bi}", name=f"T{i}_{bi}")
                    for bi in range(NBUF)
                ]
                for i in range(KH)
            ]

            nc.sync.dma_start(out=k_sb[:], in_=kprep_d[:])
            nc.vector.memset(acc[:], NEG)

            rv = imgs_d  # [c, u, b, yb, xx]

            for ci in range(C):
                for i in range(KH):
                    t = tiles[i][ci % NBUF]
                    # One rectangular load for o_local group 0: partition ys
                    # gets padded row u = ys + i (i.e. image row yb*32+ys+i-2).
                    nc.sync.dma_start(out=t[0:YS], in_=rv[ci, i : i + YS])
                    # Replicate group 0 into the other 3 o_local groups
                    # (parallel SBUF->SBUF DMAs, shallower than a log chain).
                    for g in range(1, OL):
                        nc.gpsimd.dma_start(
                            out=t[g * YS : (g + 1) * YS], in_=t[0:YS]
                        )
                for i in range(KH):
                    t = tiles[i][ci % NBUF]
                    for j in range(KW):
                        idx = (i * C + ci) * KW + j
                        nc.vector.scalar_tensor_tensor(
                            out=acc[:],
                            in0=t[:, :, :, j : j + W],
                            scalar=k_sb[:, idx : idx + 1],
                            in1=acc[:],
                            op0=mybir.AluOpType.add,
                            op1=mybir.AluOpType.max,
                        )

            for o in range(OL):
                nc.sync.dma_start(out=out_d[o], in_=acc[o * YS : (o + 1) * YS])

    nc.compile()
    return nc


def _get_program():
    if "nc" not in _CACHE:
        _CACHE["nc"] = _build_program()
    return _CACHE["nc"]


def _prep_inputs(imgs, kernel):
    imgs = np.asarray(imgs, dtype=np.float32)
    # fully padded image, -inf ring of width 2
    padded = np.full((B, C, H + 2 * PAD, W + 2 * PAD), NEG, dtype=np.float32)
    padded[:, :, PAD : PAD + H, PAD : PAD + W] = imgs
    # Y2[c, u, b, yb, x] = padded[b, c, 32*yb + u, x]
    rows = 32 * np.arange(YB)[None, :] + np.arange(YU)[:, None]  # [YU, YB]
    y2 = np.ascontiguousarray(padded[:, :, rows, :].transpose(1, 2, 0, 3, 4))
    kf = np.asarray(kernel, dtype=np.float32)[:, :, ::-1, ::-1]  # conv flip
    in_maps = []
    for m in range(NCORES):
        sl = kf[OL * m : OL * (m + 1)]  # [OL, C, KH, KW]
        # column index = (i*C + c)*KW + j  ->  order (o, i, c, j)
        tab = np.ascontiguousarray(sl.transpose(0, 2, 1, 3)).reshape(OL, NK)
        kprep = np.repeat(tab, YS, axis=0)  # [128, NK]
        in_maps.append({"imgsr": y2, "kprep": np.ascontiguousarray(kprep)})
    return in_maps


def run_spmd(imgs, kernel, trace=False):
    """Run the SPMD program; returns (full_output, BassKernelResults)."""
    from concourse.bass_utils import run_bass_kernel_spmd

    nc = _get_program()
    in_maps = _prep_inputs(imgs, kernel)
    res = run_bass_kernel_spmd(nc, in_maps, list(range(NCORES)), trace=trace)
    full = np.empty((B, O, H, W), dtype=np.float32)
    for m in range(NCORES):
        # per-core out is [OL, YS, B, YB, W]
        r = res.results[m]["out"].transpose(2, 0, 3, 1, 4)  # [B, OL, YB, YS, W]
        full[:, OL * m : OL * (m + 1)] = r.reshape(B, OL, H, W)
    return full, res


def kernel(imgs, kernel, stride=1, padding=2, dilation=1, **_ignored):
    assert int(stride) == 1 and int(padding) == 2 and int(dilation) == 1, (
        "kernel compiled for stride=1, padding=2, dilation=1"
    )
    assert tuple(imgs.shape) == (B, C, H, W), imgs.shape
    assert tuple(kernel.shape) == (O, C, KH, KW), kernel.shape
    full, _ = run_spmd(imgs, kernel, trace=False)
    return full


# revision 7
# speedup vs baseline: 811.0833x; 1.0469x over previous
"""Tropical max-plus 2D conv (BroadcastConv tropical_max) on 8 Trainium2 cores.

out[b,o,y,x] = max_{c,i,j} img_pad[b,c,y+i,x+j] + kflip[o,c,i,j]
  imgs [4,32,128,128] f32, kernel [32,32,5,5] f32, stride=1, pad=2, dil=1.

Sharding: output channels O=32 split across 8 cores (4 per core); every core
keeps the full batch so the DVE instruction free-dim is long (2048 elems).

Per-core layout:
  partitions p = o_local*32 + ys   (o_local in [0,4), ys = y % 32)
  free       = (b:4, yb:4, x)      (y = yb*32 + ys)
Host preps imgs into Y2 [c, u:36, b, yb, xx:132] with -inf padding baked into
both the 36 row-slots (u = ys + i covers shifts i in [0,5)) and the x columns,
so each of the 5 vertical kernel taps is ONE rectangular DMA into partition
group 0, replicated to the other 3 o_local groups by parallel SBUF-to-SBUF
DMAs. The 5 horizontal taps are free-dim column offsets into the x-padding.
Each (i,c,j) tap is then one fused DVE scalar_tensor_tensor instruction:
  acc = max(shifted_img + k[o,c,i,j], acc)
with the k value as a per-partition [128,1] scalar operand (k varies over the
o_local partition groups). 800 such instructions per core, FD=2048; the kernel
is DVE-throughput-bound (fp32 tensor ops are 1 elem/cycle/lane on trn2).
"""

import numpy as np

NCORES = 8
B, C, H, W = 4, 32, 128, 128
O, KH, KW = 32, 5, 5
OL = O // NCORES  # 4 output channels per core
PAD = 2
YS, YB = 32, 4  # y = yb*YS + ys
XX = W + 2 * PAD  # 132 (x-padded row)
YU = YS + 2 * PAD  # 36 padded row-slots (covers ys + shift for all 5 taps)
NK = KH * C * KW  # 800 scalar-table entries per o_local
NEG = float("-inf")

_CACHE = {}


def _build_program():
    import concourse.mybir as mybir
    from concourse import bacc
    from concourse.tile import TileContext

    f32 = mybir.dt.float32
    nc = bacc.Bacc("TRN2", target_bir_lowering=False)
    imgs_d = nc.declare_dram_parameter("imgsr", [C, YU, B, YB, XX], f32, isOutput=False)
    kprep_d = nc.declare_dram_parameter("kprep", [128, NK], f32, isOutput=False)
    out_d = nc.declare_dram_parameter("out", [OL, YS, B, YB, W], f32, isOutput=True)

    NBUF = 4  # multi-buffering depth per shift-pool

    with TileContext(nc) as tc:
        with tc.tile_pool(name="sbuf", bufs=1) as pool:
            k_sb = pool.tile([128, NK], f32, tag="ksb", name="ksb")
            acc = pool.tile([128, B, YB, W], f32, tag="acc", name="acc")
            tiles = [
                [
                    pool.tile([128, B, YB, XX], f32, tag=f"T{i}_{bi}", name=f"T{i}_{bi}")
                    for bi in range(NBUF)
                ]
                for i in range(KH)
            ]

            nc.sync.dma_start(out=k_sb[:], in_=kprep_d[:])
            nc.vector.memset(acc[:], NEG)

            rv = imgs_d  # [c, u, b, yb, xx]

            for ci in range(C):
                for i in range(KH):
                    t = tiles[i][ci % NBUF]
                    # One rectangular load for o_local group 0: partition ys
                    # gets padded row u = ys + i (i.e. image row yb*32+ys+i-2).
                    nc.sync.dma_start(out=t[0:YS], in_=rv[ci, i : i + YS])
                    # Replicate group 0 into the other 3 o_local groups
                    # (parallel SBUF->SBUF DMAs, shallower than a log chain).
                    for g in range(1, OL):
                        nc.sync.dma_start(
                            out=t[g * YS : (g + 1) * YS], in_=t[0:YS]
                        )
                for i in range(KH):
                    t = tiles[i][ci % NBUF]
                    for j in range(KW):
                        idx = (i * C + ci) * KW + j
                        nc.vector.scalar_tensor_tensor(
                            out=acc[:],
                            in0=t[:, :, :, j : j + W],
                            scalar=k_sb[:, idx : idx + 1],
                            in1=acc[:],
                            op0=mybir.AluOpType.add,
                            op1=mybir.AluOpType.max,
                        )

            for o in range(OL):
                nc.sync.dma_start(out=out_d[o], in_=acc[o * YS : (o + 1) * YS])

    nc.compile()
    return nc


def _get_program():
    if "nc" not in _CACHE:
        _CACHE["nc"] = _build_program()
    return _CACHE["nc"]


def _prep_inputs(imgs, kernel):
    imgs = np.asarray(imgs, dtype=np.float32)
    # fully padded image, -inf ring of width 2
    padded = np.full((B, C, H + 2 * PAD, W + 2 * PAD), NEG, dtype=np.float32)
    padded[:, :, PAD : PAD + H, PAD : PAD + W] = imgs
    # Y2[c, u, b, yb, x] = padded[b, c, 32*yb + u, x]
    rows = 32 * np.arange(YB)[None, :] + np.arange(YU)[:, None]  # [YU, YB]
    y2 = np.ascontiguousarray(padded[:, :, rows, :].transpose(1, 2, 0, 3, 4))
    kf = np.asarray(kernel, dtype=np.float32)[:, :, ::-1, ::-1]  # conv flip
    in_maps = []
    for m in range(NCORES):
        sl = kf[OL * m : OL * (m + 1)]  # [OL, C, KH, KW]
        # column index = (i*C + c)*KW + j  ->  order (o, i, c, j)
        tab = np.ascontiguousarray(sl.transpose(0, 2, 1, 3)).reshape(OL, NK)
        kprep = np.repeat(tab, YS, axis=0)  # [128, NK]
        in_maps.append({"imgsr": y2, "kprep": np.ascontiguousarray(kprep)})
    return in_maps


def run_spmd(imgs, kernel, trace=False):
    """Run the SPMD program; returns (full_output, BassKernelResults)."""
    from concourse.bass_utils import run_bass_kernel_spmd

    nc = _get_program()
    in_maps = _prep_inputs(imgs, kernel)
    res = run_bass_kernel_spmd(nc, in_maps, list(range(NCORES)), trace=trace)
    full = np.empty((B, O, H, W), dtype=np.float32)
    for m in range(NCORES):
        # per-core out is [OL, YS, B, YB, W]
        r = res.results[m]["out"].transpose(2, 0, 3, 1, 4)  # [B, OL, YB, YS, W]
        full[:, OL * m : OL * (m + 1)] = r.reshape(B, OL, H, W)
    return full, res


def kernel(imgs, kernel, stride=1, padding=2, dilation=1, **_ignored):
    assert int(stride) == 1 and int(padding) == 2 and int(dilation) == 1, (
        "kernel compiled for stride=1, padding=2, dilation=1"
    )
    assert tuple(imgs.shape) == (B, C, H, W), imgs.shape
    assert tuple(kernel.shape) == (O, C, KH, KW), kernel.shape
    full, _ = run_spmd(imgs, kernel, trace=False)
    return full
